# Initial kernel scaffold
#
"""Trainium2 Bass kernel for the GRU + per-joint-MLP motion predictor.

Data-parallel over 8 NeuronCores: batch 2048 -> 256 rows/core, weights
replicated.  Everything on-chip is laid out feature-major ([feature, batch])
so the recurrent state h feeds the next step's matmuls without transposes.
The GRU/recurrence path runs in float32r (FP22 multiply, fp32 accumulate,
full PE rate at N=256); the feed-forward output path (Wp / W1 / W2) runs in
bf16 so all weights stay resident in SBUF.
"""

import sys

for _p in ('/opt/trn_rl_repo/concourse', '/opt/trn_rl_repo'):
    if _p not in sys.path:
        sys.path.insert(0, _p)

import numpy as np
import ml_dtypes

import concourse.bass as bass
import concourse.mybir as mybir
import concourse.tile as tile
from concourse import bacc
from concourse.bass_utils import run_bass_kernel_spmd
from concourse.masks import make_identity

F32 = mybir.dt.float32
F32R = mybir.dt.float32r
BF16 = mybir.dt.bfloat16
AF = mybir.ActivationFunctionType
ALU = mybir.AluOpType

B, T, D = 2048, 144, 135
H = 1024
J, JD = 15, 9
SEED_LEN = 120
PRED_FRAMES = 24
NCORES = 8
BC = B // NCORES          # 256 batch rows per core
HT = H // 128             # 8 h-tiles
D0 = 128                  # first K-tile of the pose dim
D1 = D - 128              # 7 leftover pose dims


def build_program(steps=PRED_FRAMES):
    nc = bacc.Bacc(None, target_bir_lowering=False)

    x0T_in = nc.declare_dram_parameter("x0T", [D, BC], F32R, isOutput=False)
    wih_in = nc.declare_dram_parameter("wihT", [D, 3 * H], F32R, isOutput=False)
    whh_in = nc.declare_dram_parameter("whhT", [H, 3 * H], F32R, isOutput=False)
    wp_in = nc.declare_dram_parameter("wpT", [128, HT, H], BF16, isOutput=False)
    w1_in = nc.declare_dram_parameter("w1t", [J, 128, HT, 128], BF16, isOutput=False)
    w2_in = nc.declare_dram_parameter("w2bd", [J, 128, D], BF16, isOutput=False)
    bias_in = nc.declare_dram_parameter("bias", [128, 57], F32, isOutput=False)
    out_d = nc.declare_dram_parameter("out", [BC, steps, D], F32, isOutput=True)

    with tile.TileContext(nc) as tc:
        with (
            tc.tile_pool(name="wpool", bufs=1) as wpool,
            tc.tile_pool(name="hpool", bufs=15) as hpool,      # recurrent h: 2 gens x 8
            tc.tile_pool(name="longp", bufs=8) as longp,       # hb / hid: 8 live + slack
            tc.tile_pool(name="xpool", bufs=2) as xpool,       # xt0, xt1 (2 generations)
            tc.tile_pool(name="upool", bufs=2) as upool,       # u
            tc.tile_pool(name="stgp", bufs=2) as stgp,         # output staging
            tc.tile_pool(name="gate", bufs=4) as gate,         # r, z, n
            tc.tile_pool(name="tmp", bufs=3) as tmp,           # rhn, t2, d1, d2
            tc.tile_pool(name="ps", bufs=8, space="PSUM") as ps,
        ):
            # ---- resident weights ----
            wih0 = wpool.tile([128, 3 * H], F32R, tag="wih0")
            wih1 = wpool.tile([D1, 3 * H], F32R, tag="wih1")
            nc.sync.dma_start(out=wih0[:], in_=wih_in[0:128, :])
            nc.sync.dma_start(out=wih1[:], in_=wih_in[128:D, :])
            whh = []
            for k in range(HT):
                wt = wpool.tile([128, 3 * H], F32R, tag=f"whh{k}")
                nc.sync.dma_start(out=wt[:], in_=whh_in[k * 128:(k + 1) * 128, :])
                whh.append(wt)
            wpb = wpool.tile([128, HT, H], BF16, tag="wpb")
            nc.sync.dma_start(out=wpb[:], in_=wp_in[:])
            w1b = []
            for j in range(J):
                wt = wpool.tile([128, HT, 128], BF16, tag=f"w1_{j}")
                nc.sync.dma_start(out=wt[:], in_=w1_in[j])
                w1b.append(wt)
            w2one = wpool.tile([128, J, D], BF16, tag="w2")
            nc.sync.dma_start(out=w2one[:], in_=w2_in[:].rearrange("j p d -> p j d"))
            w2b = [w2one[:, j, :] for j in range(J)]

            # ---- biases (one packed tile: brz 0:16, bihn 16:24, bhhn 24:32,
            # bp 32:40, b1t 40:55, b2c 55:57) ----
            bias = wpool.tile([128, 57], F32, tag="bias")
            nc.sync.dma_start(out=bias[:], in_=bias_in[:])
            brz = bias[:, 0:16]
            bihn = bias[:, 16:24]
            bhhn = bias[:, 24:32]
            bp = bias[:, 32:40]
            b1t = bias[:, 40:55]
            b2c = bias[:, 55:57]

            # ---- identity for PE transposes (f32r to match x dtype) ----
            idf = wpool.tile([128, 128], F32, tag="idf")
            make_identity(nc, idf[:])
            ident = wpool.tile([128, 128], F32R, tag="id")
            nc.vector.tensor_copy(ident[:], idf[:])

            # ---- initial x ----
            xt0 = xpool.tile([128, BC], F32R, tag="xt0")
            xt1 = xpool.tile([D1, BC], F32R, tag="xt1")
            nc.sync.dma_start(out=xt0[:], in_=x0T_in[0:128, :])
            nc.sync.dma_start(out=xt1[:], in_=x0T_in[128:D, :])

            h_prev = None           # list of HT f32r tiles [128, BC]
            for t in range(steps):
                h_new = []
                hb_new = []
                r_tiles = []
                z_tiles = []
                for k in range(HT):
                    # --- r gate: psum = W_hh[rblk] h + W_ih[rblk] x (+bias via ACT)
                    g_r = ps.tile([128, BC], F32, tag="ps")
                    if h_prev is not None:
                        for kk in range(HT):
                            nc.tensor.matmul(
                                g_r[:], whh[kk][:, k * 128:(k + 1) * 128], h_prev[kk][:],
                                start=(kk == 0), stop=False)
                    nc.tensor.matmul(g_r[:], wih0[:, k * 128:(k + 1) * 128], xt0[:],
                                     start=(h_prev is None), stop=False)
                    nc.tensor.matmul(g_r[:], wih1[:, k * 128:(k + 1) * 128], xt1[:],
                                     start=False, stop=True)
                    r_sb = gate.tile([128, BC], F32, tag="g")
                    nc.scalar.activation(r_sb[:], g_r[:], AF.Sigmoid,
                                         bias=brz[:, k:k + 1], scale=1.0)
                    r_tiles.append(r_sb)

                    # --- z gate
                    co = H + k * 128
                    g_z = ps.tile([128, BC], F32, tag="ps")
                    if h_prev is not None:
                        for kk in range(HT):
                            nc.tensor.matmul(g_z[:], whh[kk][:, co:co + 128], h_prev[kk][:],
                                             start=(kk == 0), stop=False)
                    nc.tensor.matmul(g_z[:], wih0[:, co:co + 128], xt0[:],
                                     start=(h_prev is None), stop=False)
                    nc.tensor.matmul(g_z[:], wih1[:, co:co + 128], xt1[:],
                                     start=False, stop=True)
                    z_sb = gate.tile([128, BC], F32, tag="g")
                    nc.scalar.activation(z_sb[:], g_z[:], AF.Sigmoid,
                                         bias=brz[:, HT + k:HT + k + 1], scale=1.0)
                    z_tiles.append(z_sb)

                    # --- n gate: tanh(inn + b_ihn + r * (hn + b_hhn))
                    co = 2 * H + k * 128
                    inn = ps.tile([128, BC], F32, tag="ps")
                    nc.tensor.matmul(inn[:], wih0[:, co:co + 128], xt0[:],
                                     start=True, stop=False)
                    nc.tensor.matmul(inn[:], wih1[:, co:co + 128], xt1[:],
                                     start=False, stop=True)
                    rhn = tmp.tile([128, BC], F32, tag="ta")
                    if h_prev is not None:
                        hn = ps.tile([128, BC], F32, tag="ps")
                        for kk in range(HT):
                            nc.tensor.matmul(hn[:], whh[kk][:, co:co + 128], h_prev[kk][:],
                                             start=(kk == 0), stop=(kk == HT - 1))
                        nc.vector.scalar_tensor_tensor(
                            rhn[:], hn[:], bhhn[:, k:k + 1], r_sb[:],
                            op0=ALU.add, op1=ALU.mult)
                    else:
                        nc.vector.tensor_scalar_mul(rhn[:], r_sb[:], bhhn[:, k:k + 1])
                    t2 = tmp.tile([128, BC], F32, tag="ta")
                    nc.vector.tensor_add(t2[:], rhn[:], inn[:])
                    n_sb = gate.tile([128, BC], F32, tag="g")
                    nc.scalar.activation(n_sb[:], t2[:], AF.Tanh,
                                         bias=bihn[:, k:k + 1], scale=1.0)

                    # --- h_new = (h - n) * z + n
                    hk = hpool.tile([128, BC], F32R, tag="h")
                    if h_prev is not None:
                        d1 = tmp.tile([128, BC], F32, tag="ta")
                        nc.vector.tensor_sub(d1[:], h_prev[k][:], n_sb[:])
                        d2 = tmp.tile([128, BC], F32, tag="ta")
                        nc.vector.tensor_mul(d2[:], d1[:], z_sb[:])
                        nc.vector.tensor_add(hk[:], d2[:], n_sb[:])
                    else:
                        d2 = tmp.tile([128, BC], F32, tag="ta")
                        nc.vector.tensor_mul(d2[:], n_sb[:], z_sb[:])
                        nc.vector.tensor_sub(hk[:], n_sb[:], d2[:])
                    h_new.append(hk)
                    hbk = longp.tile([128, BC], BF16, tag="hb")
                    nc.scalar.copy(hbk[:], hk[:])
                    hb_new.append(hbk)

                # --- mlp_pre: hid = relu(Wp h + bp)   (bf16)
                hid = []
                for ko in range(HT):
                    pp = ps.tile([128, BC], F32, tag="ps")
                    for kk in range(HT):
                        nc.tensor.matmul(pp[:], wpb[:, kk, ko * 128:(ko + 1) * 128],
                                         hb_new[kk][:],
                                         start=(kk == 0), stop=(kk == HT - 1))
                    hko = longp.tile([128, BC], BF16, tag="hid")
                    nc.scalar.activation(hko[:], pp[:], AF.Relu,
                                         bias=bp[:, ko:ko + 1], scale=1.0)
                    hid.append(hko)

                # --- joint MLPs: u[j] = relu(W1[j]^T hid + b1[j]);
                # delta accumulates into dl0/dl1 interleaved per joint so each
                # u tile dies right after its W2 matmul (bounded pool use).
                dl0 = ps.tile([128, BC], F32, tag="ps")
                dl1 = None
                for j in range(J):
                    pu = ps.tile([128, BC], F32, tag="ps")
                    for kk in range(HT):
                        nc.tensor.matmul(pu[:], w1b[j][:, kk, :], hid[kk][:],
                                         start=(kk == 0), stop=(kk == HT - 1))
                    uj = upool.tile([128, BC], BF16, tag="u")
                    nc.scalar.activation(uj[:], pu[:], AF.Relu,
                                         bias=b1t[:, j:j + 1], scale=1.0)
                    nc.tensor.matmul(dl0[:], w2b[j][:, 0:128], uj[:],
                                     start=(j == 0), stop=(j == J - 1))
                    if j == J - 1:
                        dl1 = ps.tile([D1, BC], F32, tag="ps")
                        nc.tensor.matmul(dl1[:], w2b[j][:, 128:D], uj[:],
                                         start=True, stop=True)

                # --- x update (feature-major, f32r)
                nxt0 = xpool.tile([128, BC], F32R, tag="xt0")
                nc.vector.scalar_tensor_tensor(nxt0[:], dl0[:], b2c[:, 0:1], xt0[:],
                                               op0=ALU.add, op1=ALU.add)
                nxt1 = xpool.tile([D1, BC], F32R, tag="xt1")
                nc.vector.scalar_tensor_tensor(nxt1[:], dl1[:], b2c[0:D1, 1:2], xt1[:],
                                               op0=ALU.add, op1=ALU.add)
                xt0, xt1 = nxt0, nxt1

                # --- emit batch-major output rows via PE transpose
                for bt in range(2):
                    bs = slice(bt * 128, (bt + 1) * 128)
                    tp = ps.tile([128, 136], F32R, tag="ps")
                    nc.tensor.transpose(tp[:, 0:128], xt0[:, bs], ident[:])
                    # fp32r matmul dst needs an even column count: write 8
                    # cols via a [7, 8] identity slice (last col is zero).
                    nc.tensor.transpose(tp[:, 128:136], xt1[:, bs], ident[0:D1, 0:8])
                    stg = stgp.tile([128, D], F32, tag="stg")
                    nc.vector.tensor_copy(stg[:], tp[:, 0:D])
                    nc.sync.dma_start(out=out_d[bs, t, :], in_=stg[:])

                h_prev = h_new

    nc.finalize()
    return nc


def host_inputs(inputs, steps=PRED_FRAMES):
    """Full problem inputs -> per-core in_maps (host-side prep, numpy only)."""
    bf = ml_dtypes.bfloat16
    poses = np.asarray(inputs["poses"], np.float32)
    W_ih = np.asarray(inputs["W_ih"], np.float32)
    W_hh = np.asarray(inputs["W_hh"], np.float32)
    b_ih = np.asarray(inputs["b_ih"], np.float32)
    b_hh = np.asarray(inputs["b_hh"], np.float32)
    Wp = np.asarray(inputs["Wp"], np.float32)
    bp = np.asarray(inputs["bp"], np.float32)
    W1 = np.asarray(inputs["W1"], np.float32)
    b1 = np.asarray(inputs["b1"], np.float32)
    W2 = np.asarray(inputs["W2"], np.float32)
    b2 = np.asarray(inputs["b2"], np.float32)

    wihT = np.ascontiguousarray(W_ih.T)                       # [135, 3072]
    whhT = np.ascontiguousarray(W_hh.T)                       # [1024, 3072]
    wpT = np.ascontiguousarray(                               # [128, 8, 1024]
        Wp.T.reshape(HT, 128, H).transpose(1, 0, 2)).astype(bf)
    w1t = np.ascontiguousarray(                               # [15, 128, 8, 128]
        W1.reshape(J, HT, 128, 128).transpose(0, 2, 1, 3)).astype(bf)
    w2bd = np.zeros((J, 128, D), np.float32)
    for j in range(J):
        w2bd[j, :, j * JD:(j + 1) * JD] = W2[j]
    w2bd = w2bd.astype(bf)

    bias = np.zeros((128, 57), np.float32)
    bias[:, 0:16] = (b_ih + b_hh)[:2 * H].reshape(16, 128).T
    bias[:, 16:24] = b_ih[2 * H:].reshape(HT, 128).T
    bias[:, 24:32] = b_hh[2 * H:].reshape(HT, 128).T
    bias[:, 32:40] = bp.reshape(HT, 128).T
    bias[:, 40:55] = b1.T
    b2f = np.zeros(256, np.float32)
    b2f[:D] = b2.reshape(D)
    bias[:, 55:57] = b2f.reshape(2, 128).T

    shared = dict(wihT=wihT, whhT=whhT, wpT=wpT, w1t=w1t, w2bd=w2bd, bias=bias)
    x0 = poses[:, SEED_LEN - 1, :]                            # [2048, 135]
    in_maps = []
    for c in range(NCORES):
        x0T = np.ascontiguousarray(x0[c * BC:(c + 1) * BC].T)  # [135, 256]
        in_maps.append(dict(shared, x0T=x0T))
    return in_maps


_prog_cache = {}


def _get_program(steps):
    if steps not in _prog_cache:
        _prog_cache[steps] = build_program(steps)
    return _prog_cache[steps]


def run(inputs, steps=PRED_FRAMES):
    nc = _get_program(steps)
    in_maps = host_inputs(inputs, steps)
    res = run_bass_kernel_spmd(nc, in_maps, list(range(NCORES)))
    out = np.concatenate([res.results[c]["out"] for c in range(NCORES)], axis=0)
    return out


def kernel(**inputs):
    return run(inputs, PRED_FRAMES)



# revision 1
# speedup vs baseline: 680.1656x; 680.1656x over previous
"""Trainium2 Bass kernel for the GRU + per-joint-MLP motion predictor.

Data-parallel over 8 NeuronCores: batch 2048 -> 256 rows/core, weights
replicated.  Everything on-chip is laid out feature-major ([feature, batch])
so the recurrent state h feeds the next step's matmuls without transposes.
The GRU/recurrence path runs in float32r (FP22 multiply, fp32 accumulate,
full PE rate at N=256); the feed-forward output path (Wp / W1 / W2) runs in
bf16 so all weights stay resident in SBUF.
"""

import sys

for _p in ('/opt/trn_rl_repo/concourse', '/opt/trn_rl_repo'):
    if _p not in sys.path:
        sys.path.insert(0, _p)

import numpy as np
import ml_dtypes

import concourse.bass as bass
import concourse.mybir as mybir
import concourse.tile as tile
from concourse import bacc
from concourse.bass_utils import run_bass_kernel_spmd
from concourse.masks import make_identity

F32 = mybir.dt.float32
F32R = mybir.dt.float32r
BF16 = mybir.dt.bfloat16
AF = mybir.ActivationFunctionType
ALU = mybir.AluOpType

B, T, D = 2048, 144, 135
H = 1024
J, JD = 15, 9
SEED_LEN = 120
PRED_FRAMES = 24
NCORES = 8
BC = B // NCORES          # 256 batch rows per core
HT = H // 128             # 8 h-tiles
D0 = 128                  # first K-tile of the pose dim
D1 = D - 128              # 7 leftover pose dims


def build_program(steps=PRED_FRAMES):
    nc = bacc.Bacc(None, target_bir_lowering=False)

    x0T_in = nc.declare_dram_parameter("x0T", [D, BC], F32R, isOutput=False)
    wih_in = nc.declare_dram_parameter("wihT", [D, 3 * H], F32R, isOutput=False)
    whh_in = nc.declare_dram_parameter("whhT", [H, 3 * H], F32R, isOutput=False)
    wp_in = nc.declare_dram_parameter("wpT", [128, HT, H], BF16, isOutput=False)
    w1_in = nc.declare_dram_parameter("w1t", [J, 128, HT, 128], BF16, isOutput=False)
    w2_in = nc.declare_dram_parameter("w2bd", [J, 128, D], BF16, isOutput=False)
    bias_in = nc.declare_dram_parameter("bias", [128, 57], F32, isOutput=False)
    out_d = nc.declare_dram_parameter("out", [BC, steps, D], F32, isOutput=True)

    with tile.TileContext(nc) as tc:
        with (
            tc.tile_pool(name="wpool", bufs=1) as wpool,
            tc.tile_pool(name="hpool", bufs=15) as hpool,      # recurrent h: 2 gens x 8
            tc.tile_pool(name="longp", bufs=8) as longp,       # hb / hid: 8 live + slack
            tc.tile_pool(name="xpool", bufs=2) as xpool,       # xt0, xt1 (2 generations)
            tc.tile_pool(name="upool", bufs=2) as upool,       # u
            tc.tile_pool(name="stgp", bufs=2) as stgp,         # output staging
            tc.tile_pool(name="gate", bufs=4) as gate,         # r, z, n
            tc.tile_pool(name="tmp", bufs=3) as tmp,           # rhn, t2, d1, d2
            tc.tile_pool(name="ps", bufs=8, space="PSUM") as ps,
        ):
            # ---- resident weights ----
            wih0 = wpool.tile([128, 3 * H], F32R, tag="wih0")
            wih1 = wpool.tile([D1, 3 * H], F32R, tag="wih1")
            nc.sync.dma_start(out=wih0[:], in_=wih_in[0:128, :])
            nc.sync.dma_start(out=wih1[:], in_=wih_in[128:D, :])
            whh = []
            for k in range(HT):
                wt = wpool.tile([128, 3 * H], F32R, tag=f"whh{k}")
                nc.sync.dma_start(out=wt[:], in_=whh_in[k * 128:(k + 1) * 128, :])
                whh.append(wt)
            wpb = wpool.tile([128, HT, H], BF16, tag="wpb")
            nc.sync.dma_start(out=wpb[:], in_=wp_in[:])
            w1b = []
            for j in range(J):
                wt = wpool.tile([128, HT, 128], BF16, tag=f"w1_{j}")
                nc.sync.dma_start(out=wt[:], in_=w1_in[j])
                w1b.append(wt)
            w2one = wpool.tile([128, J, D], BF16, tag="w2")
            nc.sync.dma_start(out=w2one[:], in_=w2_in[:].rearrange("j p d -> p j d"))
            w2b = [w2one[:, j, :] for j in range(J)]

            # ---- biases (one packed tile: brz 0:16, bihn 16:24, bhhn 24:32,
            # bp 32:40, b1t 40:55, b2c 55:57) ----
            bias = wpool.tile([128, 57], F32, tag="bias")
            nc.sync.dma_start(out=bias[:], in_=bias_in[:])
            brz = bias[:, 0:16]
            bihn = bias[:, 16:24]
            bhhn = bias[:, 24:32]
            bp = bias[:, 32:40]
            b1t = bias[:, 40:55]
            b2c = bias[:, 55:57]

            # ---- identity for PE transposes (f32r to match x dtype) ----
            idf = wpool.tile([128, 128], F32, tag="idf")
            make_identity(nc, idf[:])
            ident = wpool.tile([128, 128], F32R, tag="id")
            nc.vector.tensor_copy(ident[:], idf[:])

            # ---- initial x ----
            xt0 = xpool.tile([128, BC], F32R, tag="xt0")
            xt1 = xpool.tile([D1, BC], F32R, tag="xt1")
            nc.sync.dma_start(out=xt0[:], in_=x0T_in[0:128, :])
            nc.sync.dma_start(out=xt1[:], in_=x0T_in[128:D, :])

            h_prev = None           # list of HT f32r tiles [128, BC]
            for t in range(steps):
                h_new = []
                hb_new = []
                r_tiles = []
                z_tiles = []
                for k in range(HT):
                    # --- r gate: psum = W_hh[rblk] h + W_ih[rblk] x (+bias via ACT)
                    g_r = ps.tile([128, BC], F32, tag="ps")
                    if h_prev is not None:
                        for kk in range(HT):
                            nc.tensor.matmul(
                                g_r[:], whh[kk][:, k * 128:(k + 1) * 128], h_prev[kk][:],
                                start=(kk == 0), stop=False)
                    nc.tensor.matmul(g_r[:], wih0[:, k * 128:(k + 1) * 128], xt0[:],
                                     start=(h_prev is None), stop=False)
                    nc.tensor.matmul(g_r[:], wih1[:, k * 128:(k + 1) * 128], xt1[:],
                                     start=False, stop=True)
                    r_sb = gate.tile([128, BC], F32, tag="g")
                    nc.scalar.activation(r_sb[:], g_r[:], AF.Sigmoid,
                                         bias=brz[:, k:k + 1], scale=1.0)
                    r_tiles.append(r_sb)

                    # --- z gate
                    co = H + k * 128
                    g_z = ps.tile([128, BC], F32, tag="ps")
                    if h_prev is not None:
                        for kk in range(HT):
                            nc.tensor.matmul(g_z[:], whh[kk][:, co:co + 128], h_prev[kk][:],
                                             start=(kk == 0), stop=False)
                    nc.tensor.matmul(g_z[:], wih0[:, co:co + 128], xt0[:],
                                     start=(h_prev is None), stop=False)
                    nc.tensor.matmul(g_z[:], wih1[:, co:co + 128], xt1[:],
                                     start=False, stop=True)
                    z_sb = gate.tile([128, BC], F32, tag="g")
                    nc.scalar.activation(z_sb[:], g_z[:], AF.Sigmoid,
                                         bias=brz[:, HT + k:HT + k + 1], scale=1.0)
                    z_tiles.append(z_sb)

                    # --- n gate: tanh(inn + b_ihn + r * (hn + b_hhn))
                    co = 2 * H + k * 128
                    inn = ps.tile([128, BC], F32, tag="ps")
                    nc.tensor.matmul(inn[:], wih0[:, co:co + 128], xt0[:],
                                     start=True, stop=False)
                    nc.tensor.matmul(inn[:], wih1[:, co:co + 128], xt1[:],
                                     start=False, stop=True)
                    rhn = tmp.tile([128, BC], F32, tag="ta")
                    if h_prev is not None:
                        hn = ps.tile([128, BC], F32, tag="ps")
                        for kk in range(HT):
                            nc.tensor.matmul(hn[:], whh[kk][:, co:co + 128], h_prev[kk][:],
                                             start=(kk == 0), stop=(kk == HT - 1))
                        nc.vector.scalar_tensor_tensor(
                            rhn[:], hn[:], bhhn[:, k:k + 1], r_sb[:],
                            op0=ALU.add, op1=ALU.mult)
                    else:
                        nc.vector.tensor_scalar_mul(rhn[:], r_sb[:], bhhn[:, k:k + 1])
                    t2 = tmp.tile([128, BC], F32, tag="ta")
                    nc.vector.tensor_add(t2[:], rhn[:], inn[:])
                    n_sb = gate.tile([128, BC], F32, tag="g")
                    nc.scalar.activation(n_sb[:], t2[:], AF.Tanh,
                                         bias=bihn[:, k:k + 1], scale=1.0)

                    # --- h_new = (h - n) * z + n
                    hk = hpool.tile([128, BC], F32R, tag="h")
                    if h_prev is not None:
                        d1 = tmp.tile([128, BC], F32, tag="ta")
                        nc.vector.tensor_sub(d1[:], h_prev[k][:], n_sb[:])
                        d2 = tmp.tile([128, BC], F32, tag="ta")
                        nc.vector.tensor_mul(d2[:], d1[:], z_sb[:])
                        nc.vector.tensor_add(hk[:], d2[:], n_sb[:])
                    else:
                        d2 = tmp.tile([128, BC], F32, tag="ta")
                        nc.vector.tensor_mul(d2[:], n_sb[:], z_sb[:])
                        nc.vector.tensor_sub(hk[:], n_sb[:], d2[:])
                    h_new.append(hk)
                    hbk = longp.tile([128, BC], BF16, tag="hb")
                    nc.scalar.copy(hbk[:], hk[:])
                    hb_new.append(hbk)

                # --- mlp_pre: hid = relu(Wp h + bp)   (bf16)
                hid = []
                for ko in range(HT):
                    pp = ps.tile([128, BC], F32, tag="ps")
                    for kk in range(HT):
                        nc.tensor.matmul(pp[:], wpb[:, kk, ko * 128:(ko + 1) * 128],
                                         hb_new[kk][:],
                                         start=(kk == 0), stop=(kk == HT - 1))
                    hko = longp.tile([128, BC], BF16, tag="hid")
                    nc.scalar.activation(hko[:], pp[:], AF.Relu,
                                         bias=bp[:, ko:ko + 1], scale=1.0)
                    hid.append(hko)

                # --- joint MLPs: u[j] = relu(W1[j]^T hid + b1[j]);
                # delta accumulates into dl0/dl1 interleaved per joint so each
                # u tile dies right after its W2 matmul (bounded pool use).
                dl0 = ps.tile([128, BC], F32, tag="ps")
                dl1 = None
                for j in range(J):
                    pu = ps.tile([128, BC], F32, tag="ps")
                    for kk in range(HT):
                        nc.tensor.matmul(pu[:], w1b[j][:, kk, :], hid[kk][:],
                                         start=(kk == 0), stop=(kk == HT - 1))
                    uj = upool.tile([128, BC], BF16, tag="u")
                    nc.scalar.activation(uj[:], pu[:], AF.Relu,
                                         bias=b1t[:, j:j + 1], scale=1.0)
                    nc.tensor.matmul(dl0[:], w2b[j][:, 0:128], uj[:],
                                     start=(j == 0), stop=(j == J - 1))
                    if j == J - 1:
                        dl1 = ps.tile([D1, BC], F32, tag="ps")
                        nc.tensor.matmul(dl1[:], w2b[j][:, 128:D], uj[:],
                                         start=True, stop=True)

                # --- x update (feature-major, f32r)
                nxt0 = xpool.tile([128, BC], F32R, tag="xt0")
                nc.vector.scalar_tensor_tensor(nxt0[:], dl0[:], b2c[:, 0:1], xt0[:],
                                               op0=ALU.add, op1=ALU.add)
                nxt1 = xpool.tile([D1, BC], F32R, tag="xt1")
                nc.vector.scalar_tensor_tensor(nxt1[:], dl1[:], b2c[0:D1, 1:2], xt1[:],
                                               op0=ALU.add, op1=ALU.add)
                xt0, xt1 = nxt0, nxt1

                # --- emit batch-major output rows via PE transpose
                for bt in range(2):
                    bs = slice(bt * 128, (bt + 1) * 128)
                    tp = ps.tile([128, 136], F32R, tag="ps")
                    nc.tensor.transpose(tp[:, 0:128], xt0[:, bs], ident[:])
                    # fp32r matmul dst needs an even column count: write 8
                    # cols via a [7, 8] identity slice (last col is zero).
                    nc.tensor.transpose(tp[:, 128:136], xt1[:, bs], ident[0:D1, 0:8])
                    stg = stgp.tile([128, D], F32, tag="stg")
                    nc.vector.tensor_copy(stg[:], tp[:, 0:D])
                    nc.sync.dma_start(out=out_d[bs, t, :], in_=stg[:])

                h_prev = h_new

    nc.finalize()
    return nc


def host_inputs(inputs, steps=PRED_FRAMES):
    """Full problem inputs -> per-core in_maps (host-side prep, numpy only)."""
    bf = ml_dtypes.bfloat16
    poses = np.asarray(inputs["poses"], np.float32)
    W_ih = np.asarray(inputs["W_ih"], np.float32)
    W_hh = np.asarray(inputs["W_hh"], np.float32)
    b_ih = np.asarray(inputs["b_ih"], np.float32)
    b_hh = np.asarray(inputs["b_hh"], np.float32)
    Wp = np.asarray(inputs["Wp"], np.float32)
    bp = np.asarray(inputs["bp"], np.float32)
    W1 = np.asarray(inputs["W1"], np.float32)
    b1 = np.asarray(inputs["b1"], np.float32)
    W2 = np.asarray(inputs["W2"], np.float32)
    b2 = np.asarray(inputs["b2"], np.float32)

    wihT = np.ascontiguousarray(W_ih.T)                       # [135, 3072]
    whhT = np.ascontiguousarray(W_hh.T)                       # [1024, 3072]
    wpT = np.ascontiguousarray(                               # [128, 8, 1024]
        Wp.T.reshape(HT, 128, H).transpose(1, 0, 2)).astype(bf)
    w1t = np.ascontiguousarray(                               # [15, 128, 8, 128]
        W1.reshape(J, HT, 128, 128).transpose(0, 2, 1, 3)).astype(bf)
    w2bd = np.zeros((J, 128, D), np.float32)
    for j in range(J):
        w2bd[j, :, j * JD:(j + 1) * JD] = W2[j]
    w2bd = w2bd.astype(bf)

    bias = np.zeros((128, 57), np.float32)
    bias[:, 0:16] = (b_ih + b_hh)[:2 * H].reshape(16, 128).T
    bias[:, 16:24] = b_ih[2 * H:].reshape(HT, 128).T
    bias[:, 24:32] = b_hh[2 * H:].reshape(HT, 128).T
    bias[:, 32:40] = bp.reshape(HT, 128).T
    bias[:, 40:55] = b1.T
    b2f = np.zeros(256, np.float32)
    b2f[:D] = b2.reshape(D)
    bias[:, 55:57] = b2f.reshape(2, 128).T

    shared = dict(wihT=wihT, whhT=whhT, wpT=wpT, w1t=w1t, w2bd=w2bd, bias=bias)
    x0 = poses[:, SEED_LEN - 1, :]                            # [2048, 135]
    in_maps = []
    for c in range(NCORES):
        x0T = np.ascontiguousarray(x0[c * BC:(c + 1) * BC].T)  # [135, 256]
        in_maps.append(dict(shared, x0T=x0T))
    return in_maps


_prog_cache = {}


def _get_program(steps):
    if steps not in _prog_cache:
        _prog_cache[steps] = build_program(steps)
    return _prog_cache[steps]


def run(inputs, steps=PRED_FRAMES):
    nc = _get_program(steps)
    in_maps = host_inputs(inputs, steps)
    res = run_bass_kernel_spmd(nc, in_maps, list(range(NCORES)))
    out = np.concatenate([res.results[c]["out"] for c in range(NCORES)], axis=0)
    return out


def kernel(**inputs):
    return run(inputs, PRED_FRAMES)

